# revision 1
# baseline (speedup 1.0000x reference)
"""ChannelAttentionPropagation1D kernel for 8x TRN2 NeuronCores.

Reference computation (per batch b):
  kv[c,d]   = sum_{t,n} key_mem[b,t,n,c] * val_mem[b,t,n,d]    # (64, 64)
  kv_soft   = softmax(kv, axis=c)
  out[n,d]  = alpha * (key_cur[b] @ kv_soft)[n,d] + val_cur[b,n,d]

Sharding (8 cores):
  phase 1: core i contracts the t=i slice of key_mem/val_mem (16384 tokens
           per batch) into a partial kv^T, then AllReduce (64 KB) over cores.
  phase 2: core i computes the n-slice [2048*i, 2048*(i+1)) of the output.

Layout notes:
  - phase 1 accumulates kvT[d,c] (PSUM) so the softmax axis c lands on the
    free axis; a tiny PE transpose afterwards yields kv_soft[c,d].
  - key_cur is transposed (and scaled by alpha) on the host so its channel
    axis is the SBUF partition axis; its token axis is permuted n = 16p + j
    so phase-2 output tiles assemble into 4KB-contiguous-per-partition
    stores.
"""

import numpy as np

import concourse.bacc as bacc
import concourse.mybir as mybir
import concourse.tile as tile
from concourse import bass_utils, masks

F32 = mybir.dt.float32

N_CORES = 8
N, T, NTOK, C, C2 = 4, 8, 16384, 64, 64
NSL = NTOK // N_CORES  # 2048: phase-2 token slice per core
A_TILES = 64           # 128-token matmul tiles per half-batch chunk
HALF = NTOK // 2       # 8192 tokens per phase-1 DMA chunk

_CACHE = {}

# Extra kwargs forwarded to run_bass_kernel_spmd (used by the profiling
# harness to request an NTFF trace; empty for normal correctness runs).
_RUN_OPTS = {}


def _build_program():
    nc = bacc.Bacc(
        "TRN2",
        target_bir_lowering=False,
        debug=False,
        enable_asserts=False,
        num_devices=N_CORES,
    )

    km = nc.dram_tensor("key_mem", [N, NTOK, C], F32, kind="ExternalInput").ap()
    vm = nc.dram_tensor("val_mem", [N, NTOK, C2], F32, kind="ExternalInput").ap()
    # key_curT is host-packed [128, NSL/2]: rows 0:64 = channels for output
    # tiles j=0..7, rows 64:128 = channels for tiles j=8..15 (row-tiled
    # phase-2 pairs).
    kct = nc.dram_tensor(
        "key_curT", [N, 128, NSL // 2], F32, kind="ExternalInput"
    ).ap()
    vc = nc.dram_tensor("val_cur", [N, NSL, C2], F32, kind="ExternalInput").ap()
    out = nc.dram_tensor("out", [N, NSL, C2], F32, kind="ExternalOutput").ap()

    with tile.TileContext(nc) as tc:
        with (
            tc.tile_pool(name="persist", bufs=1) as persist,
            tc.tile_pool(name="big", bufs=4) as big,
            tc.tile_pool(name="tmp", bufs=2) as tmp,
            tc.tile_pool(name="stage", bufs=2) as stage_pool,
            tc.tile_pool(name="ps", bufs=2, space="PSUM") as ps,
            tc.tile_pool(name="dram", bufs=1, space="DRAM") as dram,
        ):
            ident = persist.tile([128, 128], F32)
            masks.make_identity(nc, ident[:])

            kct_sb = persist.tile([128, N * (NSL // 2)], F32)
            vc_sb = persist.tile([128, N * (NSL // 128) * C2], F32)

            kvt_sb = persist.tile([C2, N * C], F32)
            kvt_all = persist.tile([C2, N * N_CORES * C], F32)
            kvt_red = persist.tile([C2, N * C], F32)
            kv_soft = persist.tile([128, N * C2], F32)
            ar_outs = {}

            def emit_tails():
                """AR readbacks + softmax + transpose + phase 2 + stores for
                all batches, emitted STAGE-MAJOR: engine FIFOs run in program
                order, so batch-major emission would serialize the four
                ~15us-latency chains. Stage-major lets the four batches
                pipeline through gpsimd/DVE/ACT/PE. All tails sit after the
                whole phase-1 so a late AllReduce (peer-core launch skew can
                exceed 100us) never blocks local phase-1 work."""
                # readbacks ride the sync queue: its chunk DMAs have drained
                # by now, while gpsimd still holds doorbell-3 (which waits
                # for the end of phase-1) and scalar holds ar_in3. Each
                # AllGather result is [rank, d, c]; pull it into SBUF as
                # [d, (rank c)] and tree-reduce with 3 DVE adds per batch.
                W = N_CORES * C
                for b in range(N):
                    nc.sync.dma_start(
                        kvt_all[:, b * W:(b + 1) * W].rearrange(
                            "d (r c) -> d r c", r=N_CORES
                        ),
                        ar_outs[b].rearrange("r d c -> d r c"),
                    )
                for width in (4 * C, 2 * C):
                    for b in range(N):
                        lo = kvt_all[:, b * W: b * W + width]
                        nc.vector.tensor_add(
                            lo, lo, kvt_all[:, b * W + width: b * W + 2 * width]
                        )
                for b in range(N):
                    nc.vector.tensor_add(
                        kvt_red[:, b * C:(b + 1) * C],
                        kvt_all[:, b * W: b * W + C],
                        kvt_all[:, b * W + C: b * W + 2 * C],
                    )
                neg_mx = tmp.tile([C2, N], F32)
                for b in range(N):
                    nc.vector.reduce_max(
                        out=neg_mx[:, b:b + 1],
                        in_=kvt_red[:, b * C:(b + 1) * C],
                        axis=mybir.AxisListType.X,
                        negate=True,
                    )
                ex = tmp.tile([C2, N * C], F32)
                sm = tmp.tile([C2, N], F32)
                for b in range(N):
                    nc.scalar.activation(
                        ex[:, b * C:(b + 1) * C],
                        kvt_red[:, b * C:(b + 1) * C],
                        mybir.ActivationFunctionType.Exp,
                        bias=neg_mx[:, b:b + 1], scale=1.0,
                        accum_out=sm[:, b:b + 1],
                    )
                rv = tmp.tile([C2, N], F32)
                for b in range(N):
                    nc.vector.reciprocal(rv[:, b:b + 1], sm[:, b:b + 1])
                for b in range(N):
                    nc.vector.tensor_scalar_mul(
                        ex[:, b * C:(b + 1) * C],
                        ex[:, b * C:(b + 1) * C],
                        rv[:, b:b + 1],
                    )
                # Transpose softmaxed kvT to kv[c, d] (transpose-mode matmul
                # must write PSUM partition 0), then mirror the whole strip
                # into partitions 64:128 with one SBUF->SBUF DMA so row-tiled
                # phase-2 can read kv from the upper rows too.
                for b in range(N):
                    tp = ps.tile([C, C2], F32, tag="tp", name=f"tp{b}", bufs=2)
                    nc.tensor.transpose(
                        tp[:], ex[:, b * C:(b + 1) * C], ident[0:C2, 0:C2]
                    )
                    nc.vector.tensor_copy(
                        kv_soft[0:C, b * C2:(b + 1) * C2], tp[:]
                    )
                nc.sync.dma_start(kv_soft[64:64 + C, :], kv_soft[0:C, :])
                stgs = {}
                for b in range(N):
                    stgs[b] = stage_pool.tile(
                        [128, (NSL // 128) * C2], F32, tag=f"stg{b}",
                        name=f"stg{b}",
                    )
                # Row-tiled phase 2: tile j contracts on PE rows 0:64
                # (kct rows 0:64, kv rows 0:64), tile j+8 on rows 64:128 —
                # the two matmuls run concurrently on separate subarrays.
                HNSL = NSL // 2
                for b in range(N):
                    for j in range(8):
                        col = slice(b * HNSL + j * 128, b * HNSL + (j + 1) * 128)
                        o_a = ps.tile(
                            [128, C2], F32, tag="o", name=f"oa{b}_{j}", bufs=4
                        )
                        nc.tensor.matmul(
                            o_a[:],
                            lhsT=kct_sb[0:C, col],
                            rhs=kv_soft[0:C, b * C2:(b + 1) * C2],
                            start=True,
                            stop=True,
                            tile_position=(0, 0),
                        )
                        o_b = ps.tile(
                            [128, C2], F32, tag="o", name=f"ob{b}_{j}", bufs=4
                        )
                        nc.tensor.matmul(
                            o_b[:],
                            lhsT=kct_sb[64:64 + C, col],
                            rhs=kv_soft[64:64 + C, b * C2:(b + 1) * C2],
                            start=True,
                            stop=True,
                            tile_position=(64, 0),
                        )
                        nc.vector.tensor_add(
                            stgs[b][:, j * C2:(j + 1) * C2],
                            o_a[:],
                            vc_sb[:, b * 1024 + j * C2: b * 1024 + (j + 1) * C2],
                        )
                        nc.vector.tensor_add(
                            stgs[b][:, (j + 8) * C2:(j + 9) * C2],
                            o_b[:],
                            vc_sb[:, b * 1024 + (j + 8) * C2: b * 1024 + (j + 9) * C2],
                        )
                    # split the store so the second half overlaps the
                    # remaining adds (trims the last batch's tail)
                    oap = out[b].rearrange("(p j) c -> p (j c)", p=128)
                    nc.sync.dma_start(oap[:, 0:8 * C2], stgs[b][:, 0:8 * C2])
                    nc.sync.dma_start(
                        oap[:, 8 * C2:16 * C2], stgs[b][:, 8 * C2:16 * C2]
                    )

            # ---- phase 1: partial kvT[d, c] per batch, col-tiled 2x ----
            # Even token-tiles accumulate on PE column group 0 (psum rows
            # 0:64), odd tiles on column group 2 (psum rows 64:128); the two
            # halves' LDWEIGHTS/MATMUL overlap on independent subarrays.
            for b in range(N):
                kv_ps = ps.tile([128, C], F32, tag="kv", name=f"kv{b}")
                for h in range(2):
                    k_sb = big.tile([128, HALF // 128 * C], F32, tag="k")
                    v_sb = big.tile([128, HALF // 128 * C2], F32, tag="v")
                    sl = slice(h * HALF, (h + 1) * HALF)
                    nc.sync.dma_start(
                        k_sb[:], km[b, sl, :].rearrange("(p a) c -> p (a c)", p=128)
                    )
                    nc.sync.dma_start(
                        v_sb[:], vm[b, sl, :].rearrange("(p a) c -> p (a c)", p=128)
                    )
                    if h == 1:
                        # phase-2 inputs for batch b: issued on the scalar
                        # (ACT) DMA FIFO so they never delay the phase-1
                        # chunk stream on the sync FIFO.
                        nc.scalar.dma_start(
                            kct_sb[:, b * (NSL // 2):(b + 1) * (NSL // 2)],
                            kct[b],
                        )
                        nc.scalar.dma_start(
                            vc_sb[:, b * 1024:(b + 1) * 1024],
                            vc[b].rearrange("(p j) c -> p (j c)", p=128),
                        )
                    for a in range(A_TILES):
                        half = a % 2
                        nc.tensor.matmul(
                            kv_ps[64 * half:64 * half + C2, :],
                            lhsT=v_sb[:, a * C2:(a + 1) * C2],
                            rhs=k_sb[:, a * C:(a + 1) * C],
                            start=(h == 0 and a < 2),
                            stop=(h == 1 and a >= A_TILES - 2),
                            tile_position=(0, 64 * half),
                        )
                # partial kvT = even-half + odd-half (DVE can read only one
                # PSUM operand per instruction, so copy then add)
                nc.vector.tensor_copy(kvt_sb[:, b * C:(b + 1) * C], kv_ps[0:C2, :])
                nc.vector.tensor_add(
                    kvt_sb[:, b * C:(b + 1) * C],
                    kvt_sb[:, b * C:(b + 1) * C],
                    kv_ps[64:64 + C2, :],
                )
                # per-batch AllGather (cheaper than AllReduce on the CC
                # core); the 8 partials are tree-reduced locally on DVE.
                ar_in = dram.tile([C2, C], F32, tag=f"ar_in{b}", name=f"ar_in{b}")
                ar_out = dram.tile(
                    [N_CORES, C2, C], F32, addr_space="Shared", tag=f"ar_out{b}",
                    name=f"ar_out{b}",
                )
                ar_outs[b] = ar_out
                nc.scalar.dma_start(ar_in[:], kvt_sb[:, b * C:(b + 1) * C])
                nc.gpsimd.collective_compute(
                    "AllGather",
                    mybir.AluOpType.bypass,
                    replica_groups=[list(range(N_CORES))],
                    ins=[ar_in.opt()],
                    outs=[ar_out.opt()],
                )
            emit_tails()

    nc.compile()
    return nc


def _get_program():
    if "nc" not in _CACHE:
        _CACHE["nc"] = _build_program()
    return _CACHE["nc"]


def kernel(key_mem, val_mem, key_cur, val_cur, alpha):
    key_mem = np.asarray(key_mem, dtype=np.float32)
    val_mem = np.asarray(val_mem, dtype=np.float32)
    key_cur = np.asarray(key_cur, dtype=np.float32)
    val_cur = np.asarray(val_cur, dtype=np.float32)
    alpha_f = float(np.asarray(alpha).reshape(-1)[0])

    nc = _get_program()

    # key_cur^T with alpha folded in; token axis permuted so that SBUF
    # column j*128+p holds token p*16+j (phase-2 store contiguity).
    kc_scaled = (alpha_f * key_cur).astype(np.float32)
    in_maps = []
    for i in range(N_CORES):
        kct_i = kc_scaled[:, i * NSL:(i + 1) * NSL, :].transpose(0, 2, 1)
        kct_i = (
            kct_i.reshape(N, C, 128, NSL // 128)
            .transpose(0, 1, 3, 2)
            .reshape(N, C, NSL)
        )
        # pack for row-tiled phase 2: rows 0:64 = tiles j=0..7,
        # rows 64:128 = tiles j=8..15
        kct_i = (
            kct_i.reshape(N, C, 2, NSL // 2)
            .transpose(0, 2, 1, 3)
            .reshape(N, 128, NSL // 2)
        )
        in_maps.append(
            {
                "key_mem": np.ascontiguousarray(key_mem[:, i]),
                "val_mem": np.ascontiguousarray(val_mem[:, i]),
                "key_curT": np.ascontiguousarray(kct_i),
                "val_cur": np.ascontiguousarray(val_cur[:, i * NSL:(i + 1) * NSL, :]),
            }
        )

    res = bass_utils.run_bass_kernel_spmd(
        nc, in_maps, core_ids=list(range(N_CORES)), **_RUN_OPTS
    )
    _CACHE["last_result"] = res
    outs = [res.results[i]["out"] for i in range(N_CORES)]
    return np.concatenate(outs, axis=1).astype(np.float32)



# revision 3
# speedup vs baseline: 1.6102x; 1.6102x over previous
"""ChannelAttentionPropagation1D kernel for 8x TRN2 NeuronCores.

Reference computation (per batch b):
  kv[c,d]   = sum_{t,n} key_mem[b,t,n,c] * val_mem[b,t,n,d]    # (64, 64)
  kv_soft   = softmax(kv, axis=c)
  out[n,d]  = alpha * (key_cur[b] @ kv_soft)[n,d] + val_cur[b,n,d]

Sharding (pair-per-batch, 8 cores):
  Core i handles batch i//2. Core 2p contracts the first half of batch
  p's T*n = 131072 memory tokens, core 2p+1 the second half; the two
  16 KB partial kv's are exchanged with a 2-core AllGather (mesh path,
  ~5us) -- the four pair-exchanges run concurrently, unlike an 8-core
  AllGather chain which serializes on the CC core. Each core then
  computes the output for its own 8192-token slice of batch p.

Precision: key/val memories, key_cur (alpha folded) and val_cur are
cast to bf16 on the host (validated rel fro err 1.7e-3 vs f64 on the
fixed inputs, 12x under the 2e-2 gate); kv accumulates in f32 PSUM and
softmax runs in f32. Halving the dominant HBM stream is worth ~55us.

Layout notes:
  - phase 1 accumulates kvT[d,c] (PSUM) so the softmax axis c lands on
    the free axis; a PE transpose afterwards yields kv_soft[c,d].
  - phase 2 keeps kv_soft stationary (weights) and STREAMS key_cur^T
    through the PE: out psum is [d, tok] so the whole output is stored
    transposed ([128, 4096] per core) and un-transposed on the host.
    Row-tiled 2x: tokens 0:4096 contract on PE quadrant (0,0), tokens
    4096:8192 on quadrant (64,0).
  - phase-1 chunk DMAs, then key_curT/val_curT, ride the sync HWDGE
    ring in program order so the phase-2 inputs never delay the
    contraction; the tiny tail DMAs (ar_in, AG readback, mirror,
    stores) ride the scalar ring and only contend at packet
    granularity.
  - the last 8192 phase-1 tokens are split into 4096/2048/1024/1024
    chunks so the final chunk's matmul tail (~8 tiles) exposes <1us
    after the last HBM byte.
"""

import numpy as np
import ml_dtypes

import concourse.bacc as bacc
import concourse.mybir as mybir
import concourse.tile as tile
from concourse import bass_utils, masks

F32 = mybir.dt.float32
BF16 = mybir.dt.bfloat16
NPBF16 = np.dtype(ml_dtypes.bfloat16)

N_CORES = 8
N, T, NTOK, C, C2 = 4, 8, 16384, 64, 64
TOT = T * NTOK // 2          # 65536 phase-1 tokens per core
NSL = 2 * NTOK // 2 // 2     # 8192 phase-2 tokens per core
HNSL = NSL // 2              # 4096 tokens per PE row-group
PAIRS = [[0, 1], [2, 3], [4, 5], [6, 7]]
# phase-1 chunk sizes (tokens); tail split keeps the exposed matmul
# chain after the last HBM byte short.
CHUNKS = [8192] * 7 + [4096, 2048, 1024, 1024]
assert sum(CHUNKS) == TOT
BLK = 512                    # phase-2 token block per psum bank

_CACHE = {}

# Extra kwargs forwarded to run_bass_kernel_spmd (used by the profiling
# harness to request an NTFF trace; empty for normal correctness runs).
_RUN_OPTS = {}


def _build_program():
    nc = bacc.Bacc(
        "TRN2",
        target_bir_lowering=False,
        debug=False,
        enable_asserts=False,
        num_devices=N_CORES,
    )

    km = nc.dram_tensor("key_mem", [TOT, C], BF16, kind="ExternalInput").ap()
    vm = nc.dram_tensor("val_mem", [TOT, C2], BF16, kind="ExternalInput").ap()
    # key_cur^T (alpha folded), row-tiled: rows 0:64 = channels x tokens
    # 0:4096, rows 64:128 = channels x tokens 4096:8192.
    kct = nc.dram_tensor("key_curT", [128, HNSL], BF16, kind="ExternalInput").ap()
    vct = nc.dram_tensor("val_curT", [128, HNSL], BF16, kind="ExternalInput").ap()
    # output, transposed: [d, tok] row-tiled the same way; host transposes.
    out = nc.dram_tensor("out", [128, HNSL], F32, kind="ExternalOutput").ap()

    with tile.TileContext(nc) as tc:
        with (
            tc.tile_pool(name="persist", bufs=1) as persist,
            tc.tile_pool(name="big", bufs=6) as big,
            tc.tile_pool(name="tmp", bufs=2) as tmp,
            tc.tile_pool(name="ps", bufs=2, space="PSUM") as ps,
            tc.tile_pool(name="dram", bufs=1, space="DRAM") as dram,
        ):
            ident = persist.tile([128, 128], F32)
            masks.make_identity(nc, ident[:])

            kct_sb = persist.tile([128, HNSL], BF16)
            vct_sb = persist.tile([128, HNSL], BF16)
            stg = persist.tile([128, HNSL], F32)
            kvt_sb = persist.tile([C2, C], F32)
            kvt_all = persist.tile([C2, 2 * C], F32)
            kvt_red = persist.tile([C2, C], F32)
            kv_soft = persist.tile([128, C2], BF16)

            # ---- phase 1: partial kvT[d, c], col-tiled 2x ----
            # Even token-tiles accumulate on PE column group 0 (psum rows
            # 0:64), odd tiles on column group 2 (psum rows 64:128).
            kv_ps = ps.tile([128, C], F32, tag="kv", bufs=1)
            n_tiles = TOT // 128
            g = 0  # global 128-token tile index
            t0 = 0
            for ci, ch in enumerate(CHUNKS):
                cols = ch // 128 * C
                k_sb = big.tile([128, 4096], BF16, tag="k")
                v_sb = big.tile([128, 4096], BF16, tag="v")
                nc.sync.dma_start(
                    k_sb[:, 0:cols],
                    km[t0:t0 + ch, :].rearrange("(p a) c -> p (a c)", p=128),
                )
                nc.sync.dma_start(
                    v_sb[:, 0:cols],
                    vm[t0:t0 + ch, :].rearrange("(p a) c -> p (a c)", p=128),
                )
                t0 += ch
                for a in range(ch // 128):
                    half = a % 2
                    nc.tensor.matmul(
                        kv_ps[64 * half:64 * half + C2, :],
                        lhsT=v_sb[:, a * C2:(a + 1) * C2],
                        rhs=k_sb[:, a * C:(a + 1) * C],
                        start=(g < 2),
                        stop=(g >= n_tiles - 2),
                        tile_position=(0, 64 * half),
                    )
                    g += 1

            # phase-2 inputs queue on the sync ring BEHIND all phase-1
            # chunks; they stream during the pair-exchange window.
            nc.sync.dma_start(kct_sb[:], kct)
            nc.sync.dma_start(vct_sb[:], vct)

            # partial kvT = even-half + odd-half (DVE reads one PSUM
            # operand per instruction: copy then add)
            nc.vector.tensor_copy(kvt_sb[:], kv_ps[0:C2, :])
            nc.vector.tensor_add(kvt_sb[:], kvt_sb[:], kv_ps[64:64 + C2, :])

            # pair exchange: 2-core AllGather (mesh path); Local outputs
            # (Shared is unsupported for <=4-core groups).
            ar_in = dram.tile([C2, C], F32, tag="ar_in", name="ar_in")
            ar_out = dram.tile([2, C2, C], F32, tag="ar_out", name="ar_out")
            nc.scalar.dma_start(ar_in[:], kvt_sb[:])
            nc.gpsimd.collective_compute(
                "AllGather",
                mybir.AluOpType.bypass,
                replica_groups=PAIRS,
                ins=[ar_in.opt()],
                outs=[ar_out.opt()],
            )
            nc.scalar.dma_start(
                kvt_all[:].rearrange("d (r c) -> d r c", r=2),
                ar_out.rearrange("r d c -> d r c"),
            )
            nc.vector.tensor_add(
                kvt_red[:], kvt_all[:, 0:C], kvt_all[:, C:2 * C]
            )

            # softmax over c (free axis)
            neg_mx = tmp.tile([C2, 1], F32)
            nc.vector.reduce_max(
                out=neg_mx[:],
                in_=kvt_red[:],
                axis=mybir.AxisListType.X,
                negate=True,
            )
            ex = tmp.tile([C2, C], F32)
            sm = tmp.tile([C2, 1], F32)
            nc.scalar.activation(
                ex[:],
                kvt_red[:],
                mybir.ActivationFunctionType.Exp,
                bias=neg_mx[:], scale=1.0,
                accum_out=sm[:],
            )
            rv = tmp.tile([C2, 1], F32)
            nc.vector.reciprocal(rv[:], sm[:])
            nc.vector.tensor_scalar_mul(ex[:], ex[:], rv[:])

            # transpose softmaxed kvT to kv[c, d] (transpose-mode matmul
            # writes PSUM partition 0), cast to bf16, and mirror into
            # partitions 64:128 for the second PE row-group.
            tp = ps.tile([C, C2], F32, tag="tp")
            nc.tensor.transpose(tp[:], ex[:], ident[0:C2, 0:C2])
            nc.vector.tensor_copy(kv_soft[0:C, :], tp[:])
            nc.scalar.dma_start(kv_soft[64:64 + C, :], kv_soft[0:C, :])

            # ---- phase 2: out^T[d, tok] = kv_soft^T @ key_cur^T ----
            # kv_soft stays stationary; kct streams 512 tokens per
            # matmul. Quadrant (0,0) covers tokens 0:4096, quadrant
            # (64,0) tokens 4096:8192, writing disjoint psum halves.
            for b in range(HNSL // BLK):
                blk = slice(b * BLK, (b + 1) * BLK)
                o = ps.tile([128, BLK], F32, tag="o", name=f"o{b}", bufs=3)
                nc.tensor.matmul(
                    o[0:C2, :],
                    lhsT=kv_soft[0:C, :],
                    rhs=kct_sb[0:C, blk],
                    start=True, stop=True,
                    tile_position=(0, 0),
                )
                nc.tensor.matmul(
                    o[64:64 + C2, :],
                    lhsT=kv_soft[64:64 + C, :],
                    rhs=kct_sb[64:64 + C, blk],
                    start=True, stop=True,
                    tile_position=(64, 64),
                )
                nc.vector.tensor_add(stg[:, blk], o[:], vct_sb[:, blk])
                nc.scalar.dma_start(out[:, blk], stg[:, blk])

    nc.compile()
    return nc


def _get_program():
    if "nc" not in _CACHE:
        _CACHE["nc"] = _build_program()
    return _CACHE["nc"]


def kernel(key_mem, val_mem, key_cur, val_cur, alpha):
    key_mem = np.asarray(key_mem, dtype=np.float32)
    val_mem = np.asarray(val_mem, dtype=np.float32)
    key_cur = np.asarray(key_cur, dtype=np.float32)
    val_cur = np.asarray(val_cur, dtype=np.float32)
    alpha_f = float(np.asarray(alpha).reshape(-1)[0])

    nc = _get_program()

    kc_scaled = (alpha_f * key_cur).astype(np.float32)
    in_maps = []
    for i in range(N_CORES):
        B, H = i // 2, i % 2
        sl = slice(H * NSL, (H + 1) * NSL)
        kct_i = (
            kc_scaled[B, sl].T
            .reshape(C, 2, HNSL).transpose(1, 0, 2).reshape(128, HNSL)
        )
        vct_i = (
            val_cur[B, sl].T
            .reshape(C2, 2, HNSL).transpose(1, 0, 2).reshape(128, HNSL)
        )
        in_maps.append(
            {
                "key_mem": np.ascontiguousarray(
                    key_mem[B, 4 * H:4 * H + 4].reshape(TOT, C)
                ).astype(NPBF16),
                "val_mem": np.ascontiguousarray(
                    val_mem[B, 4 * H:4 * H + 4].reshape(TOT, C2)
                ).astype(NPBF16),
                "key_curT": np.ascontiguousarray(kct_i).astype(NPBF16),
                "val_curT": np.ascontiguousarray(vct_i).astype(NPBF16),
            }
        )

    res = bass_utils.run_bass_kernel_spmd(
        nc, in_maps, core_ids=list(range(N_CORES)), **_RUN_OPTS
    )
    _CACHE["last_result"] = res
    out = np.empty((N, NTOK, C2), dtype=np.float32)
    for i in range(N_CORES):
        B, H = i // 2, i % 2
        o = res.results[i]["out"]  # [128, 4096] f32, row-tiled [d, tok]
        out[B, H * NSL:(H + 1) * NSL] = np.concatenate(
            [o[0:C2], o[64:64 + C2]], axis=1
        ).T
    return out


# revision 5
# speedup vs baseline: 1.6351x; 1.0155x over previous
"""ChannelAttentionPropagation1D kernel for 8x TRN2 NeuronCores.

Reference computation (per batch b):
  kv[c,d]   = sum_{t,n} key_mem[b,t,n,c] * val_mem[b,t,n,d]    # (64, 64)
  kv_soft   = softmax(kv, axis=c)
  out[n,d]  = alpha * (key_cur[b] @ kv_soft)[n,d] + val_cur[b,n,d]

Sharding (pair-per-batch, 8 cores):
  Core i handles batch i//2. Core 2p contracts the first half of batch
  p's T*n = 131072 memory tokens, core 2p+1 the second half; the two
  16 KB partial kv's are exchanged with a 2-core AllGather (mesh path)
  -- the four pair-exchanges run concurrently, unlike an 8-core
  AllGather chain which serializes on the CC core. Each core then
  computes the output for its own 8192-token slice of batch p.

Precision: key/val memories and val_cur are cast to bf16 on the host,
key_cur (alpha folded) and the softmax weights to fp8-e4m3; kv
accumulates in f32 PSUM and softmax runs in f32. Validated on the
fixed inputs: rel fro err 2.3e-3 vs f64, ~9x under the 2e-2 gate.
Halving the dominant HBM stream is worth ~55us; fp8 halves the
phase-2 PE stream.

Layout notes:
  - phase 1 accumulates kvT[d,c] (PSUM) so the softmax axis c lands on
    the free axis; a PE transpose afterwards yields kv_soft[c,d].
  - phase 2 keeps kv_soft stationary (weights) and STREAMS key_cur^T
    through the PE: out psum is [d, tok] so the whole output is stored
    transposed ([128, 4096] per core) and un-transposed on the host.
    Row-tiled 2x: tokens 0:4096 contract on PE quadrant (0,0), tokens
    4096:8192 on quadrant (64,64).
  - k chunks ride the sync HWDGE ring, v chunks the scalar ring (two
    rings double the outstanding descriptors per SDMA engine); the
    phase-2 inputs key_curT/val_curT queue at the very end of each
    ring so they never delay the contraction, streaming during the
    pair-exchange window instead. ar_in goes out on the otherwise-idle
    gpsimd SWDGE ring so it never queues; readback/mirror/stores ride
    the scalar ring, empty by then.
  - the last 8192 phase-1 tokens are split into 4096/2048/2048 chunks
    so the final chunk's matmul tail exposes <1us after the last HBM
    byte.
"""

import numpy as np
import ml_dtypes

import concourse.bacc as bacc
import concourse.mybir as mybir
import concourse.tile as tile
from concourse import bass_utils, masks

F32 = mybir.dt.float32
BF16 = mybir.dt.bfloat16
FP8 = mybir.dt.float8e4
NPBF16 = np.dtype(ml_dtypes.bfloat16)
NPFP8 = mybir.dt.np(FP8)

N_CORES = 8
N, T, NTOK, C, C2 = 4, 8, 16384, 64, 64
TOT = T * NTOK // 2          # 65536 phase-1 tokens per core
NSL = NTOK // 2              # 8192 phase-2 tokens per core
HNSL = NSL // 2              # 4096 tokens per PE row-group
PAIRS = [[0, 1], [2, 3], [4, 5], [6, 7]]
CHUNKS = [8192] * 7 + [4096, 2048, 2048]
assert sum(CHUNKS) == TOT
BLK = 512                    # phase-2 token block per psum bank
GPSIMD_ADD_BLOCKS = {1, 5}   # phase-2 add blocks offloaded to gpsimd

_CACHE = {}

# Extra kwargs forwarded to run_bass_kernel_spmd (used by the profiling
# harness to request an NTFF trace; empty for normal correctness runs).
_RUN_OPTS = {}


def _build_program():
    nc = bacc.Bacc(
        "TRN2",
        target_bir_lowering=False,
        debug=False,
        enable_asserts=False,
        num_devices=N_CORES,
    )

    km = nc.dram_tensor("key_mem", [TOT, C], BF16, kind="ExternalInput").ap()
    vm = nc.dram_tensor("val_mem", [TOT, C2], BF16, kind="ExternalInput").ap()
    # key_cur^T (alpha folded), row-tiled: rows 0:64 = channels x tokens
    # 0:4096, rows 64:128 = channels x tokens 4096:8192.
    kct = nc.dram_tensor("key_curT", [128, HNSL], FP8, kind="ExternalInput").ap()
    vct = nc.dram_tensor("val_curT", [128, HNSL], BF16, kind="ExternalInput").ap()
    # output, transposed: [d, tok] row-tiled the same way; host transposes.
    out = nc.dram_tensor("out", [128, HNSL], F32, kind="ExternalOutput").ap()

    with tile.TileContext(nc) as tc:
        with (
            tc.tile_pool(name="persist", bufs=1) as persist,
            tc.tile_pool(name="big", bufs=6) as big,
            tc.tile_pool(name="tmp", bufs=2) as tmp,
            tc.tile_pool(name="ps", bufs=2, space="PSUM") as ps,
            tc.tile_pool(name="dram", bufs=1, space="DRAM") as dram,
        ):
            ident = persist.tile([128, 128], F32)
            masks.make_identity(nc, ident[:])

            kct_sb = persist.tile([128, HNSL], FP8)
            vct_sb = persist.tile([128, HNSL], BF16)
            stg = persist.tile([128, HNSL], F32)
            kvt_sb = persist.tile([C2, C], F32)
            kvt_all = persist.tile([C2, 2 * C], F32)
            kvt_red = persist.tile([C2, C], F32)
            kv_soft = persist.tile([128, C2], FP8)

            # ---- phase 1: partial kvT[d, c], col-tiled 2x ----
            # Even token-tiles accumulate on PE column group 0 (psum rows
            # 0:64), odd tiles on column group 2 (psum rows 64:128).
            kv_ps = ps.tile([128, C], F32, tag="kv", bufs=1)
            n_tiles = TOT // 128
            g = 0  # global 128-token tile index
            t0 = 0
            for ci, ch in enumerate(CHUNKS):
                cols = ch // 128 * C
                k_sb = big.tile([128, 4096], BF16, tag="k")
                v_sb = big.tile([128, 4096], BF16, tag="v")
                nc.sync.dma_start(
                    k_sb[:, 0:cols],
                    km[t0:t0 + ch, :].rearrange("(p a) c -> p (a c)", p=128),
                )
                nc.scalar.dma_start(
                    v_sb[:, 0:cols],
                    vm[t0:t0 + ch, :].rearrange("(p a) c -> p (a c)", p=128),
                )
                t0 += ch
                for a in range(ch // 128):
                    half = a % 2
                    nc.tensor.matmul(
                        kv_ps[64 * half:64 * half + C2, :],
                        lhsT=v_sb[:, a * C2:(a + 1) * C2],
                        rhs=k_sb[:, a * C:(a + 1) * C],
                        start=(g < 2),
                        stop=(g >= n_tiles - 2),
                        tile_position=(0, 64 * half),
                    )
                    g += 1

            # phase-2 inputs queue BEHIND the phase-1 chunks on each ring;
            # they stream during the pair-exchange window.
            nc.sync.dma_start(kct_sb[:], kct)
            nc.scalar.dma_start(vct_sb[:], vct)

            # partial kvT = even-half + odd-half (DVE reads one PSUM
            # operand per instruction: copy then add)
            nc.vector.tensor_copy(kvt_sb[:], kv_ps[0:C2, :])
            nc.vector.tensor_add(kvt_sb[:], kvt_sb[:], kv_ps[64:64 + C2, :])

            # pair exchange: 2-core AllGather (mesh path); Local outputs
            # (Shared is unsupported for <=4-core groups). ar_in rides the
            # idle gpsimd SWDGE ring so it never queues behind kct/vct.
            ar_in = dram.tile([C2, C], F32, tag="ar_in", name="ar_in")
            ar_out = dram.tile([2, C2, C], F32, tag="ar_out", name="ar_out")
            nc.gpsimd.dma_start(ar_in[:], kvt_sb[:])
            nc.gpsimd.collective_compute(
                "AllGather",
                mybir.AluOpType.bypass,
                replica_groups=PAIRS,
                ins=[ar_in.opt()],
                outs=[ar_out.opt()],
            )
            nc.scalar.dma_start(
                kvt_all[:].rearrange("d (r c) -> d r c", r=2),
                ar_out.rearrange("r d c -> d r c"),
            )
            nc.vector.tensor_add(
                kvt_red[:], kvt_all[:, 0:C], kvt_all[:, C:2 * C]
            )

            # softmax over c (free axis)
            neg_mx = tmp.tile([C2, 1], F32)
            nc.vector.reduce_max(
                out=neg_mx[:],
                in_=kvt_red[:],
                axis=mybir.AxisListType.X,
                negate=True,
            )
            ex = tmp.tile([C2, C], F32)
            sm = tmp.tile([C2, 1], F32)
            nc.scalar.activation(
                ex[:],
                kvt_red[:],
                mybir.ActivationFunctionType.Exp,
                bias=neg_mx[:], scale=1.0,
                accum_out=sm[:],
            )
            rv = tmp.tile([C2, 1], F32)
            nc.vector.reciprocal(rv[:], sm[:])
            nc.vector.tensor_scalar_mul(ex[:], ex[:], rv[:])

            # transpose softmaxed kvT to kv[c, d] (transpose-mode matmul
            # writes PSUM partition 0), cast to fp8, and mirror into
            # partitions 64:128 for the second PE quadrant.
            tp = ps.tile([C, C2], F32, tag="tp")
            nc.tensor.transpose(tp[:], ex[:], ident[0:C2, 0:C2])
            nc.vector.tensor_copy(kv_soft[0:C, :], tp[:])
            nc.scalar.dma_start(kv_soft[64:64 + C, :], kv_soft[0:C, :])

            # ---- phase 2: out^T[d, tok] = kv_soft^T @ key_cur^T ----
            # kv_soft stays stationary; kct streams 512 fp8 tokens per
            # matmul. Quadrant (0,0) covers tokens 0:4096, quadrant
            # (64,64) tokens 4096:8192, writing disjoint psum halves.
            for b in range(HNSL // BLK):
                blk = slice(b * BLK, (b + 1) * BLK)
                o = ps.tile([128, BLK], F32, tag="o", name=f"o{b}", bufs=3)
                nc.tensor.matmul(
                    o[0:C2, :],
                    lhsT=kv_soft[0:C, :],
                    rhs=kct_sb[0:C, blk],
                    start=True, stop=True,
                    tile_position=(0, 0),
                )
                nc.tensor.matmul(
                    o[64:64 + C2, :],
                    lhsT=kv_soft[64:64 + C, :],
                    rhs=kct_sb[64:64 + C, blk],
                    start=True, stop=True,
                    tile_position=(64, 64),
                )
                nc.vector.tensor_add(stg[:, blk], o[:], vct_sb[:, blk])
                nc.scalar.dma_start(out[:, blk], stg[:, blk])

    nc.compile()
    return nc


def _get_program():
    if "nc" not in _CACHE:
        _CACHE["nc"] = _build_program()
    return _CACHE["nc"]


def kernel(key_mem, val_mem, key_cur, val_cur, alpha):
    key_mem = np.asarray(key_mem, dtype=np.float32)
    val_mem = np.asarray(val_mem, dtype=np.float32)
    key_cur = np.asarray(key_cur, dtype=np.float32)
    val_cur = np.asarray(val_cur, dtype=np.float32)
    alpha_f = float(np.asarray(alpha).reshape(-1)[0])

    nc = _get_program()

    kc_scaled = (alpha_f * key_cur).astype(np.float32)
    in_maps = []
    for i in range(N_CORES):
        B, H = i // 2, i % 2
        sl = slice(H * NSL, (H + 1) * NSL)
        kct_i = (
            kc_scaled[B, sl].T
            .reshape(C, 2, HNSL).transpose(1, 0, 2).reshape(128, HNSL)
        )
        vct_i = (
            val_cur[B, sl].T
            .reshape(C2, 2, HNSL).transpose(1, 0, 2).reshape(128, HNSL)
        )
        in_maps.append(
            {
                "key_mem": np.ascontiguousarray(
                    key_mem[B, 4 * H:4 * H + 4].reshape(TOT, C)
                ).astype(NPBF16),
                "val_mem": np.ascontiguousarray(
                    val_mem[B, 4 * H:4 * H + 4].reshape(TOT, C2)
                ).astype(NPBF16),
                "key_curT": np.ascontiguousarray(kct_i).astype(NPFP8),
                "val_curT": np.ascontiguousarray(vct_i).astype(NPBF16),
            }
        )

    res = bass_utils.run_bass_kernel_spmd(
        nc, in_maps, core_ids=list(range(N_CORES)), **_RUN_OPTS
    )
    _CACHE["last_result"] = res
    out = np.empty((N, NTOK, C2), dtype=np.float32)
    for i in range(N_CORES):
        B, H = i // 2, i % 2
        o = res.results[i]["out"]  # [128, 4096] f32, row-tiled [d, tok]
        out[B, H * NSL:(H + 1) * NSL] = np.concatenate(
            [o[0:C2], o[64:64 + C2]], axis=1
        ).T
    return out
